# revision 4
# baseline (speedup 1.0000x reference)
"""AFT-Full forward on 8 Trainium2 NeuronCores.

Sharding: core c -> (batch b = c//2, output-time-half h = c%2).
Each core computes out[b, h*1024:(h+1)*1024, :] with no cross-core
communication. Host-side work is only layout prep (transpose / roll /
dtype cast) and the final gather.

Per-core math (T=2048, D=1024, H=256, Th=1024 = this core's t-half):
  K|V   = x_b @ [Wk|Wv]              [T, 512]   (bf16 matmul, f32 psum)
  eK    = exp(K + bk), eKV = eK*(V + bv)        stored [s, h] in SBUF
  Q^T   = Wq^T @ x_b^T[:, t-half]    [H, Th]    (bf16 matmul)
  sQ    = sigmoid(Q^T + bq)
  den^T = eK^T-accum:  sum_s eK[s,h] * ew^T[s,t]   (fp32r matmul)
  num^T = same with eKV                            (fp32r matmul)
  Yt^T  = sQ * num^T / den^T
  out^T = Wp^T @ Yt^T + bp           [D, Th]    (fp32r matmul)

The t-axis of x^T and the s-axis of wbias^T are rolled by -h*1024 per
core so "this core's t-half" is always columns 0:1024 of the rolled
frame; sums over s are order-invariant so the roll is harmless.
"""

import sys

for _p in ("/opt/trn_rl_repo",):
    if _p not in sys.path:
        sys.path.insert(0, _p)

import numpy as np
import ml_dtypes

import concourse.bacc as bacc
import concourse.tile as tile
from concourse import mybir
from concourse.bass_utils import run_bass_kernel_spmd

BF16 = ml_dtypes.bfloat16

B, T, DIM, HID = 4, 2048, 1024, 256
TH = T // 2          # per-core t-half
N_CORES = 8
P = 128              # partitions
ND = DIM // P        # 8 d-tiles
NT = T // P          # 16 t(/s)-tiles
NH = HID // P        # 2 h-tiles
NM = DIM // P        # 8 output dim-tiles
CH = 512             # matmul moving free-dim chunk
F32 = mybir.dt.float32
F32R = mybir.dt.float32r
DBF = mybir.dt.bfloat16


def _build():
    nc = bacc.Bacc(None, target_bir_lowering=False)

    xt_ext = nc.declare_dram_parameter("xt", [DIM, T], DBF, isOutput=False)
    wq_ext = nc.declare_dram_parameter("wq", [DIM, HID], DBF, isOutput=False)
    wkv_ext = nc.declare_dram_parameter("wkv", [DIM, 2 * HID], DBF, isOutput=False)
    wp_ext = nc.declare_dram_parameter("wp", [HID, DIM], F32R, isOutput=False)
    wbt_ext = nc.declare_dram_parameter("wbt", [T, TH], DBF, isOutput=False)
    bq_ext = nc.declare_dram_parameter("bq2", [P, NH], F32, isOutput=False)
    bkv_ext = nc.declare_dram_parameter("bkv", [P, 2 * HID], F32, isOutput=False)
    bp_ext = nc.declare_dram_parameter("bp8", [P, NM], F32, isOutput=False)
    out_ext = nc.declare_dram_parameter("outT", [DIM, TH], F32, isOutput=True)

    with tile.TileContext(nc) as tc:
        with (
            tc.tile_pool(name="persist", bufs=1) as pp,
            tc.tile_pool(name="stream", bufs=3) as sp,
            tc.tile_pool(name="evac", bufs=3) as ep,
        ):
            # ---- resident SBUF tensors ----
            xt = pp.tile([P, ND, T], DBF, tag="xt")
            wq = pp.tile([P, ND, HID], DBF, tag="wq")
            wkv = pp.tile([P, ND, 2 * HID], DBF, tag="wkv")
            wp = pp.tile([P, NH, DIM], F32R, tag="wp")
            bq2 = pp.tile([P, NH], F32, tag="bq2")
            bkv = pp.tile([P, 2 * HID], F32, tag="bkv")
            bp8 = pp.tile([P, NM], F32, tag="bp8")
            ekvk = pp.tile([P, NT, 2 * HID], F32R, tag="ekvk")  # eK | eKV per s-tile
            sq = pp.tile([P, NH, TH], F32, tag="sq")
            yt = pp.tile([P, NH, TH], F32R, tag="yt")

            for n in range(ND):
                nc.sync.dma_start(wq[:, n, :], wq_ext[n * P:(n + 1) * P, :])
                nc.sync.dma_start(wkv[:, n, :], wkv_ext[n * P:(n + 1) * P, :])
            for u in range(NH):
                nc.sync.dma_start(wp[:, u, :], wp_ext[u * P:(u + 1) * P, :])
            nc.sync.dma_start(bq2[:, :], bq_ext[:, :])
            nc.sync.dma_start(bkv[:, :], bkv_ext[:, :])
            nc.sync.dma_start(bp8[:, :], bp_ext[:, :])

            # x^T loaded in [128d, 512t] chunks so K/V matmuls can start early
            for j in range(T // CH):
                for n in range(ND):
                    nc.sync.dma_start(
                        xt[:, n, j * CH:(j + 1) * CH],
                        xt_ext[n * P:(n + 1) * P, j * CH:(j + 1) * CH],
                    )

            # ---- phase 1a: K|V = x @ [Wk|Wv], then eK / eKV ----
            with tc.tile_pool(name="ps1", bufs=3, space="PSUM") as ps1:
                for i in range(NT):
                    pkv = ps1.tile([P, 2 * HID], F32, tag="pkv")
                    for n in range(ND):
                        nc.tensor.matmul(
                            pkv[:, :],
                            xt[:, n, i * P:(i + 1) * P],
                            wkv[:, n, :],
                            start=(n == 0),
                            stop=(n == ND - 1),
                        )
                    kvb = sp.tile([P, 2 * HID], F32, tag="kvb")
                    nc.vector.tensor_add(kvb[:, :], pkv[:, :], bkv[:, :])
                    # eK = exp(K + bk)
                    nc.scalar.activation(
                        ekvk[:, i, 0:HID], kvb[:, 0:HID],
                        mybir.ActivationFunctionType.Exp,
                    )
                    # eKV = eK * (V + bv)
                    nc.vector.tensor_mul(
                        ekvk[:, i, HID:2 * HID], ekvk[:, i, 0:HID],
                        kvb[:, HID:2 * HID],
                    )

                # ---- phase 1b: Q^T for this core's t-half ----
                for u in range(NH):
                    pqt = ps1.tile([P, TH], F32, tag="pqt", bufs=2)
                    for c in range(TH // CH):
                        for n in range(ND):
                            nc.tensor.matmul(
                                pqt[:, c * CH:(c + 1) * CH],
                                wq[:, n, u * P:(u + 1) * P],
                                xt[:, n, c * CH:(c + 1) * CH],
                                start=(n == 0),
                                stop=(n == ND - 1),
                            )
                    nc.scalar.activation(
                        sq[:, u, :], pqt[:, :],
                        mybir.ActivationFunctionType.Sigmoid,
                        bias=bq2[:, u:u + 1],
                    )

            # ---- phase 2: den^T / num^T over s, fp32r ----
            with tc.tile_pool(name="ps2", bufs=1, space="PSUM") as ps2:
                accs = [
                    ps2.tile([P, TH], F32, tag=f"acc{a}", name=f"acc{a}")
                    for a in range(4)
                ]
                # acc0/1 = den^T h-tile 0/1, acc2/3 = num^T h-tile 0/1
                for st in range(NT):
                    wbt = sp.tile([P, TH], DBF, tag="wbt")
                    nc.sync.dma_start(wbt[:, :], wbt_ext[st * P:(st + 1) * P, :])
                    ew = sp.tile([P, TH], F32R, tag="ew")
                    nc.scalar.activation(
                        ew[:, :], wbt[:, :], mybir.ActivationFunctionType.Exp
                    )
                    for a in range(4):
                        u = a % 2
                        base = (a // 2) * HID  # 0 -> eK(den), HID -> eKV(num)
                        lh = ekvk[:, st, base + u * P: base + (u + 1) * P]
                        for c in range(TH // CH):
                            nc.tensor.matmul(
                                accs[a][:, c * CH:(c + 1) * CH],
                                lh,
                                ew[:, c * CH:(c + 1) * CH],
                                start=(st == 0),
                                stop=(st == NT - 1),
                            )
                # Yt^T = sQ * num^T / den^T
                for u in range(NH):
                    rec = sp.tile([P, TH], F32, tag="rec")
                    nc.vector.reciprocal(rec[:, :], accs[u][:, :])
                    tmp = sp.tile([P, TH], F32, tag="tmp")
                    nc.vector.tensor_mul(tmp[:, :], accs[2 + u][:, :], rec[:, :])
                    nc.vector.tensor_mul(yt[:, u, :], tmp[:, :], sq[:, u, :])

            # ---- phase 3: out^T = Wp^T @ Yt^T + bp ----
            with tc.tile_pool(name="ps3", bufs=2, space="PSUM") as ps3:
                for m in range(NM):
                    po = ps3.tile([P, TH], F32, tag="po")
                    for c in range(TH // CH):
                        for u in range(NH):
                            nc.tensor.matmul(
                                po[:, c * CH:(c + 1) * CH],
                                wp[:, u, m * P:(m + 1) * P],
                                yt[:, u, c * CH:(c + 1) * CH],
                                start=(u == 0),
                                stop=(u == NH - 1),
                            )
                    ob = ep.tile([P, TH], F32, tag="ob")
                    nc.scalar.add(ob[:, :], po[:, :], bp8[:, m:m + 1])
                    nc.sync.dma_start(out_ext[m * P:(m + 1) * P, :], ob[:, :])

    nc.finalize()
    return nc


_NC = None


def _get_nc():
    global _NC
    if _NC is None:
        _NC = _build()
    return _NC


def _make_in_maps(x, Wq, bq, Wk, bk, Wv, bv, Wp, bp, wbias):
    wq = np.ascontiguousarray(Wq, dtype=np.float32).astype(BF16)
    wkv = np.concatenate([Wk, Wv], axis=1).astype(np.float32).astype(BF16)
    wp = np.ascontiguousarray(Wp, dtype=np.float32)
    bq2 = np.ascontiguousarray(np.asarray(bq, np.float32).reshape(NH, P).T)
    bkv_row = np.concatenate([bk, bv]).astype(np.float32)
    bkv = np.ascontiguousarray(np.broadcast_to(bkv_row, (P, 2 * HID)))
    bp8 = np.ascontiguousarray(np.asarray(bp, np.float32).reshape(NM, P).T)
    wb = np.asarray(wbias, np.float32)[:T, :T]

    in_maps = []
    for c in range(N_CORES):
        b, half = divmod(c, 2)
        toff = half * TH
        xt = np.roll(np.asarray(x[b], np.float32).T, -toff, axis=1)
        xt = np.ascontiguousarray(xt).astype(BF16)
        # ew^T[s_rolled, j] = wbias[toff + j, (s_rolled + toff) % T]
        wbt = np.roll(wb[toff:toff + TH, :], -toff, axis=1).T
        wbt = np.ascontiguousarray(wbt).astype(BF16)
        in_maps.append({
            "xt": xt, "wq": wq, "wkv": wkv, "wp": wp, "wbt": wbt,
            "bq2": bq2, "bkv": bkv, "bp8": bp8,
        })
    return in_maps


def run_on_hw(in_maps, trace=False):
    nc = _get_nc()
    return run_bass_kernel_spmd(
        nc, in_maps, core_ids=list(range(N_CORES)), trace=trace
    )


def kernel(**inputs) -> np.ndarray:
    in_maps = _make_in_maps(**inputs)
    res = run_on_hw(in_maps, trace=False)
    out = np.empty((B, T, DIM), dtype=np.float32)
    for c in range(N_CORES):
        b, half = divmod(c, 2)
        toff = half * TH
        out[b, toff:toff + TH, :] = res.results[c]["outT"].T
    return out


# revision 5
# speedup vs baseline: 1.0104x; 1.0104x over previous
"""AFT-Full forward on 8 Trainium2 NeuronCores.

Sharding: core c -> (batch b = c//2, output-time-half h = c%2).
Each core computes out[b, h*1024:(h+1)*1024, :] with no cross-core
communication. Host-side work is only layout prep (transpose / roll /
dtype cast) and the final gather.

Per-core math (T=2048, D=1024, H=256, Th=1024 = this core's t-half):
  Q^T   = Wq^T @ x_b^T[:, t-half]    [H, Th]    (bf16 matmul)
  sQ    = sigmoid(Q^T + bq)
  K|V   = x_b @ [Wk|Wv]              [T, 512]   (bf16 matmul, f32 psum)
  eK    = exp(K + bk), eKV = eK*(V + bv)        stored [s, h] in SBUF
  den^T = sum_s eK[s,h] * ew^T[s,t]             (fp32r matmul)
  num^T = same with eKV                         (fp32r matmul)
  Yt^T  = sQ * num^T / den^T
  out^T = Wp^T @ Yt^T + bp           [D, Th]    (fp32r matmul)

The t-axis of x^T and the s-axis of wbias^T are rolled by -h*1024 per
core so "this core's t-half" is always columns 0:1024 of the rolled
frame; sums over s are order-invariant so the roll is harmless.
"""

import sys

for _p in ("/opt/trn_rl_repo",):
    if _p not in sys.path:
        sys.path.insert(0, _p)

import numpy as np
import ml_dtypes

import concourse.bacc as bacc
import concourse.tile as tile
from concourse import mybir
from concourse.bass_utils import run_bass_kernel_spmd

BF16 = ml_dtypes.bfloat16

B, T, DIM, HID = 4, 2048, 1024, 256
TH = T // 2          # per-core t-half
N_CORES = 8
P = 128              # partitions
ND = DIM // P        # 8 d-tiles
NT = T // P          # 16 t(/s)-tiles
NH = HID // P        # 2 h-tiles
NM = DIM // P        # 8 output dim-tiles
CH = 512             # matmul moving free-dim chunk
NC_CH = TH // CH     # 2 chunks per t-half
WBG = 4              # wbias s-tiles per batched DMA
F32 = mybir.dt.float32
F32R = mybir.dt.float32r
DBF = mybir.dt.bfloat16
AF = mybir.ActivationFunctionType


def _build():
    nc = bacc.Bacc(None, target_bir_lowering=False)

    xt_ext = nc.declare_dram_parameter("xt", [DIM, T], DBF, isOutput=False)
    wq_ext = nc.declare_dram_parameter("wq", [DIM, HID], DBF, isOutput=False)
    wkv_ext = nc.declare_dram_parameter("wkv", [DIM, 2 * HID], DBF, isOutput=False)
    wp_ext = nc.declare_dram_parameter("wp", [HID, DIM], F32R, isOutput=False)
    wbt_ext = nc.declare_dram_parameter("wbt", [T, TH], DBF, isOutput=False)
    bias_ext = nc.declare_dram_parameter("bias", [P, 522], F32, isOutput=False)
    out_ext = nc.declare_dram_parameter("outT", [DIM, TH], F32, isOutput=True)

    xt_r = xt_ext.rearrange("(n p) t -> p n t", p=P)
    wq_r = wq_ext.rearrange("(n p) h -> p n h", p=P)
    wkv_r = wkv_ext.rearrange("(n p) h -> p n h", p=P)
    wp_r = wp_ext.rearrange("(u p) m -> p u m", p=P)
    wbt_r = wbt_ext.rearrange("(g p) t -> p g t", p=P)

    with tile.TileContext(nc) as tc:
        with (
            tc.tile_pool(name="persist", bufs=1) as pp,
            tc.tile_pool(name="stream", bufs=3) as sp,
            tc.tile_pool(name="evac", bufs=3) as ep,
        ):
            # ---- resident SBUF tensors ----
            xt = pp.tile([P, ND, T], DBF, tag="xt")
            wq = pp.tile([P, ND, HID], DBF, tag="wq")
            wkv = pp.tile([P, ND, 2 * HID], DBF, tag="wkv")
            wp = pp.tile([P, NH, DIM], F32R, tag="wp")
            bias = pp.tile([P, 522], F32, tag="bias")
            ekvk = pp.tile([P, NT, 2 * HID], F32R, tag="ekvk")  # eK | eKV
            sq = pp.tile([P, NH, TH], F32, tag="sq")
            yt = pp.tile([P, NH, TH], F32R, tag="yt")
            bq2 = bias[:, 0:NH]
            bkv = bias[:, NH:NH + 2 * HID]
            bp8 = bias[:, NH + 2 * HID:522]

            # ---- DMAs, ordered by first use (HWDGE FIFO) ----
            nc.sync.dma_start(wq[:, :, :], wq_r[:, :, :])
            nc.sync.dma_start(xt[:, :, 0:CH], xt_r[:, :, 0:CH])
            nc.sync.dma_start(xt[:, :, CH:2 * CH], xt_r[:, :, CH:2 * CH])
            nc.sync.dma_start(bias[:, :], bias_ext[:, :])
            nc.sync.dma_start(wkv[:, :, :], wkv_r[:, :, :])
            nc.sync.dma_start(xt[:, :, 2 * CH:3 * CH], xt_r[:, :, 2 * CH:3 * CH])
            nc.sync.dma_start(xt[:, :, 3 * CH:4 * CH], xt_r[:, :, 3 * CH:4 * CH])
            # wbias^T batches prefetch behind x
            wbts = []
            for g in range(NT // WBG):
                wbt = sp.tile([P, WBG, TH], DBF, tag="wbt", bufs=2)
                nc.sync.dma_start(
                    wbt[:, :, :], wbt_r[:, g * WBG:(g + 1) * WBG, :]
                )
                wbts.append(wbt)
            nc.sync.dma_start(wp[:, :, :], wp_r[:, :, :])

            # ---- phase 1a: Q^T = Wq^T @ x^T[:, 0:TH] ----
            with tc.tile_pool(name="ps1", bufs=1, space="PSUM") as ps1:
                for u in range(NH):
                    pqt = ps1.tile([P, TH], F32, tag=f"pqt{u}", name=f"pqt{u}")
                    for c in range(NC_CH):
                        for n in range(ND):
                            nc.tensor.matmul(
                                pqt[:, c * CH:(c + 1) * CH],
                                wq[:, n, u * P:(u + 1) * P],
                                xt[:, n, c * CH:(c + 1) * CH],
                                start=(n == 0),
                                stop=(n == ND - 1),
                            )
                    nc.scalar.activation(
                        sq[:, u, :], pqt[:, :], AF.Sigmoid,
                        bias=bq2[:, u:u + 1],
                    )

                # ---- phase 1b: K|V, eK, eKV ----
                for i in range(NT):
                    pkv = ps1.tile([P, 2 * HID], F32, tag="pkv", bufs=3)
                    for n in range(ND):
                        nc.tensor.matmul(
                            pkv[:, :],
                            xt[:, n, i * P:(i + 1) * P],
                            wkv[:, n, :],
                            start=(n == 0),
                            stop=(n == ND - 1),
                        )
                    kvb = sp.tile([P, 2 * HID], F32, tag="kvb")
                    nc.vector.tensor_add(kvb[:, :], pkv[:, :], bkv[:, :])
                    nc.scalar.activation(
                        ekvk[:, i, 0:HID], kvb[:, 0:HID], AF.Exp
                    )
                    nc.vector.tensor_mul(
                        ekvk[:, i, HID:2 * HID], ekvk[:, i, 0:HID],
                        kvb[:, HID:2 * HID],
                    )

            # ---- phase 2: den^T (acc0/1) and num^T (acc2/3) ----
            with tc.tile_pool(name="ps2", bufs=1, space="PSUM") as ps2:
                accs = [
                    ps2.tile([P, TH], F32, tag=f"acc{a}", name=f"acc{a}")
                    for a in range(4)
                ]
                for st in range(NT):
                    ew = sp.tile([P, TH], F32R, tag="ew")
                    nc.scalar.activation(
                        ew[:, :], wbts[st // WBG][:, st % WBG, :], AF.Exp
                    )
                    for a in range(4):
                        u = a % 2
                        base = (a // 2) * HID  # 0 -> eK(den), HID -> eKV(num)
                        lh = ekvk[:, st, base + u * P: base + (u + 1) * P]
                        for c in range(NC_CH):
                            nc.tensor.matmul(
                                accs[a][:, c * CH:(c + 1) * CH],
                                lh,
                                ew[:, c * CH:(c + 1) * CH],
                                start=(st == 0),
                                stop=(st == NT - 1),
                            )

                # ---- epilogue: Yt^T = sQ * num^T / den^T (chunked) ----
                # recips first so den slots (acc0/1) free early for phase 3
                recs = {}
                for u in range(NH):
                    for c in range(NC_CH):
                        r = sp.tile([P, CH], F32, tag="rec", bufs=4,
                                    name=f"rec{u}{c}")
                        nc.vector.reciprocal_approx_fast(
                            r[:, :], accs[u][:, c * CH:(c + 1) * CH]
                        )
                        recs[(u, c)] = r
                for c in range(NC_CH):
                    for u in range(NH):
                        cs = slice(c * CH, (c + 1) * CH)
                        tmp = sp.tile([P, CH], F32, tag="tmp")
                        nc.vector.tensor_mul(tmp[:, :], accs[2 + u][:, cs],
                                             recs[(u, c)][:, :])
                        nc.vector.tensor_mul(yt[:, u, cs], tmp[:, :],
                                             sq[:, u, cs])

                # ---- phase 3: out^T = Wp^T @ Yt^T + bp ----
                # psum slots recycle acc0/acc1 (released after the recips)
                for c in range(NC_CH):
                    for m in range(NM):
                        po = ps2.tile([P, CH], F32, tag=f"acc{m % 2}",
                                      name=f"po{c}{m}")
                        for u in range(NH):
                            nc.tensor.matmul(
                                po[:, :],
                                wp[:, u, m * P:(m + 1) * P],
                                yt[:, u, c * CH:(c + 1) * CH],
                                start=(u == 0),
                                stop=(u == NH - 1),
                            )
                        ob = ep.tile([P, CH], F32, tag="ob")
                        nc.scalar.add(ob[:, :], po[:, :], bp8[:, m:m + 1])
                        nc.sync.dma_start(
                            out_ext[m * P:(m + 1) * P, c * CH:(c + 1) * CH],
                            ob[:, :],
                        )

    nc.finalize()
    return nc


_NC = None


def _get_nc():
    global _NC
    if _NC is None:
        _NC = _build()
    return _NC


def _make_in_maps(x, Wq, bq, Wk, bk, Wv, bv, Wp, bp, wbias):
    wq = np.ascontiguousarray(Wq, dtype=np.float32).astype(BF16)
    wkv = np.concatenate([Wk, Wv], axis=1).astype(np.float32).astype(BF16)
    wp = np.ascontiguousarray(Wp, dtype=np.float32)
    bias = np.zeros((P, 522), np.float32)
    bias[:, 0:NH] = np.asarray(bq, np.float32).reshape(NH, P).T
    bias[:, NH:NH + 2 * HID] = np.concatenate([bk, bv]).astype(np.float32)
    bias[:, NH + 2 * HID:] = np.asarray(bp, np.float32).reshape(NM, P).T
    wb = np.asarray(wbias, np.float32)[:T, :T]

    in_maps = []
    for c in range(N_CORES):
        b, half = divmod(c, 2)
        toff = half * TH
        xt = np.roll(np.asarray(x[b], np.float32).T, -toff, axis=1)
        xt = np.ascontiguousarray(xt).astype(BF16)
        # ew^T[s_rolled, j] = wbias[toff + j, (s_rolled + toff) % T]
        wbt = np.roll(wb[toff:toff + TH, :], -toff, axis=1).T
        wbt = np.ascontiguousarray(wbt).astype(BF16)
        in_maps.append({
            "xt": xt, "wq": wq, "wkv": wkv, "wp": wp, "wbt": wbt,
            "bias": bias,
        })
    return in_maps


def run_on_hw(in_maps, trace=False):
    nc = _get_nc()
    return run_bass_kernel_spmd(
        nc, in_maps, core_ids=list(range(N_CORES)), trace=trace
    )


def kernel(**inputs) -> np.ndarray:
    in_maps = _make_in_maps(**inputs)
    res = run_on_hw(in_maps, trace=False)
    out = np.empty((B, T, DIM), dtype=np.float32)
    for c in range(N_CORES):
        b, half = divmod(c, 2)
        toff = half * TH
        out[b, toff:toff + TH, :] = res.results[c]["outT"].T
    return out


# revision 6
# speedup vs baseline: 1.0844x; 1.0732x over previous
"""AFT-Full forward on 8 Trainium2 NeuronCores.

Sharding: core c -> (batch b = c//2, output-time-half h = c%2).
Each core computes out[b, h*1024:(h+1)*1024, :] with no cross-core
communication. Host-side work is only layout prep (transpose / roll /
dtype cast) and the final gather.

Per-core math (T=2048, D=1024, H=256, Th=1024 = this core's t-half):
  Q^T   = Wq^T @ x_b^T[:, t-half]    [H, Th]    (bf16 matmul)
  sQ    = sigmoid(Q^T + bq)
  K|V   = x_b @ [Wk|Wv]              [T, 512]   (bf16 matmul, f32 psum)
  eK    = exp(K + bk), eKV = eK*(V + bv)        stored [s, h] in SBUF
  den^T = sum_s eK[s,h] * ew^T[s,t]             (fp32r matmul)
  num^T = same with eKV                         (fp32r matmul)
  Yt^T  = sQ * num^T / den^T
  out^T = Wp^T @ Yt^T + bp           [D, Th]    (fp32r matmul)

The t-axis of x^T and the s-axis of wbias^T are rolled by -h*1024 per
core so "this core's t-half" is always columns 0:1024 of the rolled
frame; sums over s are order-invariant so the roll is harmless.
"""

import sys

for _p in ("/opt/trn_rl_repo",):
    if _p not in sys.path:
        sys.path.insert(0, _p)

import numpy as np
import ml_dtypes

import concourse.bacc as bacc
import concourse.tile as tile
from concourse import mybir
from concourse.bass_utils import run_bass_kernel_spmd

BF16 = ml_dtypes.bfloat16

B, T, DIM, HID = 4, 2048, 1024, 256
TH = T // 2          # per-core t-half
N_CORES = 8
P = 128              # partitions
ND = DIM // P        # 8 d-tiles
NT = T // P          # 16 t(/s)-tiles
NH = HID // P        # 2 h-tiles
NM = DIM // P        # 8 output dim-tiles
CH = 512             # matmul moving free-dim chunk
NC_CH = TH // CH     # 2 chunks per t-half
WBG = 4              # wbias s-tiles per batched DMA
F32 = mybir.dt.float32
F32R = mybir.dt.float32r
DBF = mybir.dt.bfloat16
AF = mybir.ActivationFunctionType


def _build():
    nc = bacc.Bacc(None, target_bir_lowering=False)

    xt_ext = nc.declare_dram_parameter("xt", [DIM, T], DBF, isOutput=False)
    wq_ext = nc.declare_dram_parameter("wq", [DIM, HID], DBF, isOutput=False)
    wkv_ext = nc.declare_dram_parameter("wkv", [DIM, 2 * HID], DBF, isOutput=False)
    wp_ext = nc.declare_dram_parameter("wp", [HID, DIM], F32R, isOutput=False)
    wbt_ext = nc.declare_dram_parameter("wbt", [T, TH], DBF, isOutput=False)
    bias_ext = nc.declare_dram_parameter("bias", [P, 522], F32, isOutput=False)
    out_ext = nc.declare_dram_parameter("outT", [DIM, TH], F32, isOutput=True)

    xt_r = xt_ext.rearrange("(n p) t -> p n t", p=P)
    wq_r = wq_ext.rearrange("(n p) h -> p n h", p=P)
    wkv_r = wkv_ext.rearrange("(n p) h -> p n h", p=P)
    wp_r = wp_ext.rearrange("(u p) m -> p u m", p=P)
    wbt_r = wbt_ext.rearrange("(g p) t -> p g t", p=P)

    with tile.TileContext(nc) as tc:
        with (
            tc.tile_pool(name="persist", bufs=1) as pp,
            tc.tile_pool(name="stream", bufs=3) as sp,
            tc.tile_pool(name="evac", bufs=3) as ep,
        ):
            # ---- resident SBUF tensors ----
            xt = pp.tile([P, ND, T], DBF, tag="xt")
            wq = pp.tile([P, ND, HID], DBF, tag="wq")
            wkv = pp.tile([P, ND, 2 * HID], DBF, tag="wkv")
            wp = pp.tile([P, NH, DIM], F32R, tag="wp")
            bias = pp.tile([P, 522], F32, tag="bias")
            ekvk = pp.tile([P, NT, 2 * HID], F32R, tag="ekvk")  # eK | eKV
            sq = pp.tile([P, NH, TH], F32, tag="sq")
            yt = pp.tile([P, NH, TH], F32R, tag="yt")
            bq2 = bias[:, 0:NH]
            bkv = bias[:, NH:NH + 2 * HID]
            bp8 = bias[:, NH + 2 * HID:522]

            # ---- DMAs, ordered by first use (HWDGE FIFO on sync) ----
            nc.sync.dma_start(wq[:, :, :], wq_r[:, :, :])
            for n in range(ND):
                nc.sync.dma_start(xt[:, n, :], xt_r[:, n, :])
            nc.sync.dma_start(bias[:, :], bias_ext[:, :])
            nc.sync.dma_start(wkv[:, :, :], wkv_r[:, :, :])
            nc.sync.dma_start(wp[:, :, :], wp_r[:, :, :])
            # wbias^T batches go through the idle GpSimd SWDGE queue so
            # their descriptor generation runs parallel to the sync FIFO
            wbts = []
            for g in range(NT // WBG):
                wbt = sp.tile([P, WBG, TH], DBF, tag="wbt", bufs=2)
                nc.gpsimd.dma_start(
                    wbt[:, :, :], wbt_r[:, g * WBG:(g + 1) * WBG, :]
                )
                wbts.append(wbt)

            # ---- phase 1a: Q^T = Wq^T @ x^T[:, 0:TH] ----
            with tc.tile_pool(name="ps1", bufs=1, space="PSUM") as ps1:
                pqts = [
                    ps1.tile([P, TH], F32, tag=f"pqt{u}", name=f"pqt{u}")
                    for u in range(NH)
                ]
                for n in range(ND):
                    for u in range(NH):
                        for c in range(NC_CH):
                            nc.tensor.matmul(
                                pqts[u][:, c * CH:(c + 1) * CH],
                                wq[:, n, u * P:(u + 1) * P],
                                xt[:, n, c * CH:(c + 1) * CH],
                                start=(n == 0),
                                stop=(n == ND - 1),
                            )
                for u in range(NH):
                    nc.scalar.activation(
                        sq[:, u, :], pqts[u][:, :], AF.Sigmoid,
                        bias=bq2[:, u:u + 1],
                    )

                # pre-exp the first two ew tiles so phase 2 can start
                # the moment the last K/V tile lands (ACT is FIFO)
                ews = {}
                for st in range(2):
                    ew = sp.tile([P, TH], F32R, tag="ew", name=f"ew{st}")
                    nc.scalar.activation(
                        ew[:, :], wbts[st // WBG][:, st % WBG, :], AF.Exp
                    )
                    ews[st] = ew

                # ---- phase 1b: K|V, eK, eKV ----
                for i in range(NT):
                    pkv = ps1.tile([P, 2 * HID], F32, tag="pkv", bufs=3)
                    for n in range(ND):
                        nc.tensor.matmul(
                            pkv[:, :],
                            xt[:, n, i * P:(i + 1) * P],
                            wkv[:, n, :],
                            start=(n == 0),
                            stop=(n == ND - 1),
                        )
                    kvb = sp.tile([P, 2 * HID], F32, tag="kvb")
                    nc.vector.tensor_add(kvb[:, :], pkv[:, :], bkv[:, :])
                    nc.scalar.activation(
                        ekvk[:, i, 0:HID], kvb[:, 0:HID], AF.Exp
                    )
                    nc.vector.tensor_mul(
                        ekvk[:, i, HID:2 * HID], ekvk[:, i, 0:HID],
                        kvb[:, HID:2 * HID],
                    )

            # ---- phase 2: den^T (acc0/1) and num^T (acc2/3) ----
            with tc.tile_pool(name="ps2", bufs=1, space="PSUM") as ps2:
                accs = [
                    ps2.tile([P, TH], F32, tag=f"acc{a}", name=f"acc{a}")
                    for a in range(4)
                ]
                for st in range(NT):
                    if st in ews:
                        ew = ews[st]
                    else:
                        ew = sp.tile([P, TH], F32R, tag="ew", name=f"ew{st}")
                        nc.scalar.activation(
                            ew[:, :], wbts[st // WBG][:, st % WBG, :], AF.Exp
                        )
                    for a in range(4):
                        u = a % 2
                        base = (a // 2) * HID  # 0 -> eK(den), HID -> eKV(num)
                        lh = ekvk[:, st, base + u * P: base + (u + 1) * P]
                        for c in range(NC_CH):
                            nc.tensor.matmul(
                                accs[a][:, c * CH:(c + 1) * CH],
                                lh,
                                ew[:, c * CH:(c + 1) * CH],
                                start=(st == 0),
                                stop=(st == NT - 1),
                            )

                # ---- epilogue: Yt^T = sQ * num^T / den^T (chunked) ----
                # recips first so den slots (acc0/1) free early for phase 3
                recs = {}
                for u in range(NH):
                    for c in range(NC_CH):
                        r = sp.tile([P, CH], F32, tag="rec", bufs=4,
                                    name=f"rec{u}{c}")
                        nc.vector.reciprocal_approx_fast(
                            r[:, :], accs[u][:, c * CH:(c + 1) * CH]
                        )
                        recs[(u, c)] = r
                for c in range(NC_CH):
                    for u in range(NH):
                        cs = slice(c * CH, (c + 1) * CH)
                        tmp = sp.tile([P, CH], F32, tag="tmp")
                        nc.vector.tensor_mul(tmp[:, :], accs[2 + u][:, cs],
                                             recs[(u, c)][:, :])
                        nc.vector.tensor_mul(yt[:, u, cs], tmp[:, :],
                                             sq[:, u, cs])

                # ---- phase 3: out^T = Wp^T @ Yt^T + bp ----
                # psum slots recycle acc0/acc1 (released after the recips)
                out_r = out_ext.rearrange("(m p) t -> p m t", p=P)
                OG = 4  # m-tiles per staged output DMA
                for c in range(NC_CH):
                    for mg in range(NM // OG):
                        ob = ep.tile([P, OG, CH], F32, tag="ob", bufs=2,
                                     name=f"ob{c}{mg}")
                        for k in range(OG):
                            m = mg * OG + k
                            po = ps2.tile([P, CH], F32, tag=f"acc{m % 2}",
                                          name=f"po{c}{m}")
                            for u in range(NH):
                                nc.tensor.matmul(
                                    po[:, :],
                                    wp[:, u, m * P:(m + 1) * P],
                                    yt[:, u, c * CH:(c + 1) * CH],
                                    start=(u == 0),
                                    stop=(u == NH - 1),
                                )
                            if m % 2 == 0:
                                nc.scalar.add(ob[:, k, :], po[:, :],
                                              bp8[:, m:m + 1])
                            else:
                                nc.vector.tensor_scalar_add(
                                    ob[:, k, :], po[:, :], bp8[:, m:m + 1]
                                )
                        nc.sync.dma_start(
                            out_r[:, mg * OG:(mg + 1) * OG,
                                  c * CH:(c + 1) * CH],
                            ob[:, :, :],
                        )

    nc.finalize()
    return nc


_NC = None


def _get_nc():
    global _NC
    if _NC is None:
        _NC = _build()
    return _NC


def _make_in_maps(x, Wq, bq, Wk, bk, Wv, bv, Wp, bp, wbias):
    wq = np.ascontiguousarray(Wq, dtype=np.float32).astype(BF16)
    wkv = np.concatenate([Wk, Wv], axis=1).astype(np.float32).astype(BF16)
    wp = np.ascontiguousarray(Wp, dtype=np.float32)
    bias = np.zeros((P, 522), np.float32)
    bias[:, 0:NH] = np.asarray(bq, np.float32).reshape(NH, P).T
    bias[:, NH:NH + 2 * HID] = np.concatenate([bk, bv]).astype(np.float32)
    bias[:, NH + 2 * HID:] = np.asarray(bp, np.float32).reshape(NM, P).T
    wb = np.asarray(wbias, np.float32)[:T, :T]

    in_maps = []
    for c in range(N_CORES):
        b, half = divmod(c, 2)
        toff = half * TH
        xt = np.roll(np.asarray(x[b], np.float32).T, -toff, axis=1)
        xt = np.ascontiguousarray(xt).astype(BF16)
        # ew^T[s_rolled, j] = wbias[toff + j, (s_rolled + toff) % T]
        wbt = np.roll(wb[toff:toff + TH, :], -toff, axis=1).T
        wbt = np.ascontiguousarray(wbt).astype(BF16)
        in_maps.append({
            "xt": xt, "wq": wq, "wkv": wkv, "wp": wp, "wbt": wbt,
            "bias": bias,
        })
    return in_maps


def run_on_hw(in_maps, trace=False):
    nc = _get_nc()
    return run_bass_kernel_spmd(
        nc, in_maps, core_ids=list(range(N_CORES)), trace=trace
    )


def kernel(**inputs) -> np.ndarray:
    in_maps = _make_in_maps(**inputs)
    res = run_on_hw(in_maps, trace=False)
    out = np.empty((B, T, DIM), dtype=np.float32)
    for c in range(N_CORES):
        b, half = divmod(c, 2)
        toff = half * TH
        out[b, toff:toff + TH, :] = res.results[c]["outT"].T
    return out
